# revision 41
# baseline (speedup 1.0000x reference)
"""Trainium2 Bass kernel for nn_CausalSelfAttention_5411658793445.

Sharding: queries (token dim) split 8 ways; K/V projection also token-split,
with the current block's roped K / V exchanged via one fused AllGather so
every core attends over the full kept KV window.

v2 changes vs the 607us baseline (trace-driven):
  - exp batching: score l-tiles are matmul'd into a 5-bank PSUM tile and
    exponentiated by ONE ACT instruction over a strided [128, 5, 330] AP.
    The ACT engine has a ~352-cycle fixed cost per instruction; the baseline
    paid it per 330-element tile (579 ns avg, 313 us total).  Grouped, the
    amortized cost is ~400 ns/tile.
  - dense KV packing: kept KV = prior tail (2640) + current (2640) = 5280,
    padded to 5376 (42 l-tiles) instead of the baseline's 5760 (-6.7% PE/ACT
    work in attention, -14% collective bytes).
  - one fused AllGather for K and V instead of two.
  - host-side weight / prior-KV re-layout so every big DMA is one
    contiguous descriptor per partition.
  - divides moved from ACT (Act.Copy w/ scale) to the Vector engine.
"""

import math
from contextlib import ExitStack

import numpy as np
import ml_dtypes

NC = 8
DIM, NH, HD = 1536, 12, 128
HALF = 64
H, W = 22, 40
FRAME = H * W            # 880
S_TOTAL = 2640
SC = S_TOTAL // NC       # 330
ST = 110                 # s-subtile (330 = 3*110)
EPS = 1e-6
CT = 22
CH = 21
CW = 21
NK = DIM // 128          # 12 contraction chunks
USE_SCH = True           # Vector-engine Schraudolph exp for B-groups

_BF16 = ml_dtypes.bfloat16
_cache: dict = {}


def _build_theta(freqs_angle, cs):
    start_frame = cs // FRAME
    nf = S_TOTAL // FRAME
    t = freqs_angle[start_frame:start_frame + nf, :CT]
    h = freqs_angle[:H, CT:CT + CH]
    w = freqs_angle[:W, CT + CH:CT + CH + CW]
    tf = np.broadcast_to(t[:, None, None, :], (nf, H, W, CT))
    hf = np.broadcast_to(h[None, :, None, :], (nf, H, W, CH))
    wf = np.broadcast_to(w[None, None, :, :], (nf, H, W, CW))
    return np.concatenate([tf, hf, wf], axis=-1).reshape(nf * H * W, HALF)


def _build_program(n_prior, l_pad):
    import concourse.bass as bass  # noqa: F401
    import concourse.tile as tile
    from concourse import bacc, mybir
    from concourse.masks import make_identity

    f32 = mybir.dt.float32
    bf16 = mybir.dt.bfloat16
    i16 = mybir.dt.int16
    Act = mybir.ActivationFunctionType
    Alu = mybir.AluOpType

    NLT = l_pad // 128                    # total l-tiles (42)
    L1T = n_prior // 128                  # pure-prior l-tiles (20)
    P1C = L1T * 128                       # pass-1 cols (2560)
    TAIL = n_prior - P1C                  # prior tail cols in pass 2 (80)
    P2C = l_pad - P1C                     # pass-2 cols (2816)
    L2T = P2C // 128                      # pass-2 l-tiles (22)
    NPAD = l_pad - (n_prior + S_TOTAL)    # zero-pad kv rows (96)
    sm_scale = 1.0 / math.sqrt(HD)
    CCN = HD * SC                         # flat K or V block per head (42240)

    nc = bacc.Bacc("TRN2", target_bir_lowering=False, debug=False,
                   num_devices=NC)

    # host-prepped layouts: every big DMA is contiguous per partition
    xs_d = nc.dram_tensor("xs_d", [128, NK, SC], bf16, kind="ExternalInput").ap()
    thetaT = nc.dram_tensor("thetaT", [HALF, SC], f32, kind="ExternalInput").ap()
    wq = nc.dram_tensor("wq", [128, NH, NK, 128], bf16, kind="ExternalInput").ap()
    wk = nc.dram_tensor("wk", [128, NH, NK, 128], bf16, kind="ExternalInput").ap()
    wv = nc.dram_tensor("wv", [128, NK, 3, 512], bf16, kind="ExternalInput").ap()
    wo = nc.dram_tensor("wo", [128, NK, 3, 512], bf16, kind="ExternalInput").ap()
    bq2 = nc.dram_tensor("bq2", [HD, NH], f32, kind="ExternalInput").ap()
    bk2 = nc.dram_tensor("bk2", [HD, NH], f32, kind="ExternalInput").ap()
    gq2 = nc.dram_tensor("gq2", [HD, NH], f32, kind="ExternalInput").ap()
    gk2 = nc.dram_tensor("gk2", [HD, NH], f32, kind="ExternalInput").ap()
    bv1 = nc.dram_tensor("bv1", [1, DIM], bf16, kind="ExternalInput").ap()
    bo1 = nc.dram_tensor("bo1", [1, DIM], bf16, kind="ExternalInput").ap()
    pswT = nc.dram_tensor("pswT", [HD, HD], bf16, kind="ExternalInput").ap()
    priorKT = nc.dram_tensor("priorKT", [NH, HD, n_prior], bf16,
                             kind="ExternalInput").ap()
    # prior V pre-laid as [h, p, lt, 130]  (lt*128+p = kept row index;
    # cols 128:130 pre-filled 1.0 so the load is one run per partition)
    priorVP = nc.dram_tensor("priorVP", [NH, 128, L1T + 1, 130], bf16,
                             kind="ExternalInput").ap()
    out = nc.dram_tensor("out", [SC, DIM], f32, kind="ExternalOutput").ap()

    with tile.TileContext(nc, trace_sim=False) as tc, ExitStack() as ctx:
        consts = ctx.enter_context(tc.tile_pool(name="consts", bufs=1))
        wstr = ctx.enter_context(tc.tile_pool(name="wstr", bufs=3))
        xpool = ctx.enter_context(tc.tile_pool(name="xpool", bufs=1))
        acts = ctx.enter_context(tc.tile_pool(name="acts", bufs=1))
        sqp = ctx.enter_context(tc.tile_pool(name="sqp", bufs=2))
        csrp = ctx.enter_context(tc.tile_pool(name="csrp", bufs=2))
        kvs = ctx.enter_context(tc.tile_pool(name="kvs", bufs=2))
        escp = ctx.enter_context(tc.tile_pool(name="escp", bufs=4))
        smal = ctx.enter_context(tc.tile_pool(name="smal", bufs=4))
        outp = ctx.enter_context(tc.tile_pool(name="outp", bufs=1))
        dram = ctx.enter_context(tc.tile_pool(name="dram", bufs=1, space="DRAM"))
        # PSUM: psc (scA 3 banks + scB 2 banks) + pav (2x1) + pb (1) = 8
        psc = ctx.enter_context(tc.tile_pool(name="psc", bufs=1, space="PSUM"))
        pav = ctx.enter_context(tc.tile_pool(name="pav", bufs=2, space="PSUM"))
        pb = ctx.enter_context(tc.tile_pool(name="pb", bufs=1, space="PSUM"))

        # ---------- constants ----------
        _constv_cache = {}

        def constv(val):
            if val not in _constv_cache:
                t = consts.tile([128, 1], f32, name=f"cv_{len(_constv_cache)}")
                nc.vector.memset(t, val)
                _constv_cache[val] = t
            return _constv_cache[val]

        # x first: the first projection matmul depends only on this + wm0
        xs = xpool.tile([128, NK, SC], bf16)
        nc.sync.dma_start(xs, xs_d)

        ident = consts.tile([128, 128], f32)
        make_identity(nc, ident)
        ones_col = consts.tile([128, 1], f32)
        nc.vector.memset(ones_col, 1.0)
        ones_row = consts.tile([1, 128], bf16)
        nc.vector.memset(ones_row, 1.0)
        ones_row_f = consts.tile([1, 128], f32)
        nc.vector.memset(ones_row_f, 1.0)
        psw_sb = consts.tile([HD, HD], bf16)
        nc.sync.dma_start(psw_sb, pswT)
        th2 = consts.tile([128, SC], f32)
        nc.sync.dma_start(th2[0:HALF, :], thetaT)
        nc.sync.dma_start(th2[HALF:128, :], thetaT)
        # CC = [cos; cos], SS = [-sin; sin]
        cc = consts.tile([128, SC], f32)
        ss = consts.tile([128, SC], f32)
        nc.scalar.activation(cc, th2, Act.Sin, bias=constv(math.pi / 2.0))
        nc.scalar.activation(ss[0:HALF, :], th2[0:HALF, :], Act.Sin,
                             scale=constv(-1.0)[0:HALF])
        nc.scalar.activation(ss[HALF:128, :], th2[HALF:128, :], Act.Sin)
        bq_sb = consts.tile([HD, NH], f32)
        bk_sb = consts.tile([HD, NH], f32)
        gq_sb = consts.tile([HD, NH], f32)
        gk_sb = consts.tile([HD, NH], f32)
        nc.sync.dma_start(bq_sb, bq2)
        nc.sync.dma_start(bk_sb, bk2)
        nc.sync.dma_start(gq_sb, gq2)
        nc.sync.dma_start(gk_sb, gk2)
        bqg = consts.tile([HD, NH], f32)
        bkg = consts.tile([HD, NH], f32)
        nc.vector.tensor_mul(bqg, bq_sb, gq_sb)
        nc.vector.tensor_mul(bkg, bk_sb, gk_sb)
        bv_sb = consts.tile([1, DIM], bf16)
        bo_sb = consts.tile([1, DIM], bf16)
        nc.sync.dma_start(bv_sb, bv1)
        nc.sync.dma_start(bo_sb, bo1)

        # ---------- internal DRAM for the K / V collectives ----------
        k_cc_in = dram.tile([NH, CCN], bf16)
        v_cc_in = dram.tile([NH, CCN], bf16)
        kg = dram.tile([NC, NH, CCN], bf16, addr_space="Shared")
        vg = dram.tile([NC, NH, CCN], bf16, addr_space="Shared")
        rgroups = [list(range(NC))]

        # ---------- projection helper (q / k): [d, t] + norm factors ------
        def qk_projection(w_dram, b_sb, g_sb, bg_sb, name):
            raw = acts.tile([128, NH, SC], bf16, tag=f"raw_{name}")
            pss = pb.tile([128, SC], f32, tag="pb", name=f"pss_{name}")
            for m in range(NH):
                wm = wstr.tile([128, NK, 128], bf16, tag="wm",
                               name=f"wm_{name}_{m}")
                nc.sync.dma_start(wm, w_dram[:, m])
                ps = pav.tile([128, 512], f32, tag="pav", name=f"pj_{name}_{m}")
                for kk in range(NK):
                    nc.tensor.matmul(
                        ps[:, :SC], wm[:, kk, :], xs[:, kk, :],
                        start=(kk == 0), stop=(kk == NK - 1))
                nc.scalar.activation(raw[:, m, :], ps[:, :SC], Act.Identity,
                                     bias=bg_sb[:, m:m + 1],
                                     scale=g_sb[:, m:m + 1])
                sq = sqp.tile([128, SC], f32, tag="sq")
                nc.scalar.activation(sq, ps[:, :SC], Act.Square,
                                     bias=b_sb[:, m:m + 1])
                nc.tensor.matmul(pss[0:1, :], ones_col, sq,
                                 start=(m == 0), stop=(m == NH - 1))
            r1 = smal.tile([1, SC], f32, tag="r1")
            nc.scalar.activation(r1, pss[0:1, :], Act.Sqrt,
                                 scale=constv(1.0 / DIM)[0:1],
                                 bias=constv(EPS)[0:1])
            rr = smal.tile([1, SC], f32, tag="rr")
            nc.vector.reciprocal(rr, r1)
            rrb = psc.tile([128, 512], f32, tag="scB", name=f"rrb_{name}")
            nc.tensor.matmul(rrb[:, :SC], ones_row_f, rr,
                             start=True, stop=True)
            ccr = csrp.tile([128, SC], f32, tag="ccr")
            ssr = csrp.tile([128, SC], f32, tag="ssr")
            nc.vector.tensor_mul(ccr, cc, rrb[:, :SC])
            nc.vector.tensor_mul(ssr, ss, rrb[:, :SC])
            return raw, ccr, ssr

        def rope_chunk(raw, ccr, ssr, m, dst_ap, name):
            # dst = raw*ccr + swap_halves(raw)*ssr   (swap via PE matmul)
            pw = pb.tile([128, 512], f32, tag="pb", name=f"sw_{name}_{m}")
            nc.tensor.matmul(pw[:, :SC], psw_sb, raw[:, m, :],
                             start=True, stop=True)
            m1 = sqp.tile([128, SC], f32, tag="m1")
            nc.vector.tensor_mul(m1, raw[:, m, :], ccr)
            m2 = sqp.tile([128, SC], f32, tag="m2")
            nc.vector.tensor_mul(m2, pw[:, :SC], ssr)
            nc.vector.tensor_add(dst_ap, m1, m2)

        # ---------- K ----------
        raw_k, ccr_k, ssr_k = qk_projection(wk, bk_sb, gk_sb, bkg, "k")
        kn = acts.tile([128, NH, SC], bf16, tag="kn")
        for m in range(NH):
            rope_chunk(raw_k, ccr_k, ssr_k, m, kn[:, m, :], "k")
        for m in range(NH):
            nc.sync.dma_start(
                k_cc_in[m].rearrange("(d t) -> d t", d=128), kn[:, m, :])
        nc.gpsimd.collective_compute(
            "AllGather", Alu.bypass, replica_groups=rgroups,
            ins=[k_cc_in.opt()], outs=[kg.opt()])

        # ---------- V (direct [t, d] production) ----------
        vt = acts.tile([128, 3, DIM], bf16, tag="vt")
        for oc in range(3):
            pvt = psc.tile([128, 3, 512], f32, tag="scA", name=f"pv_{oc}")
            for kk in range(NK):
                wc = wstr.tile([128, 512], bf16, tag="wc", name=f"wv_{oc}_{kk}")
                nc.sync.dma_start(wc, wv[:, kk, oc])
                for tci in range(3):
                    nc.tensor.matmul(
                        pvt[:ST, tci, :],
                        xs[:, kk, tci * ST:(tci + 1) * ST], wc,
                        start=(kk == 0), stop=False)
            for tci in range(3):
                nc.tensor.matmul(
                    pvt[:ST, tci, :], ones_row[:, :ST],
                    bv_sb[:, oc * 512:(oc + 1) * 512],
                    start=False, stop=True)
                nc.vector.tensor_copy(
                    vt[:ST, tci, oc * 512:(oc + 1) * 512], pvt[:ST, tci, :])
        for h in range(NH):
            nc.sync.dma_start(
                v_cc_in[h].rearrange("(tc p d) -> p tc d", p=ST, d=HD),
                vt[:ST, :, h * 128:(h + 1) * 128])
        nc.gpsimd.collective_compute(
            "AllGather", Alu.bypass, replica_groups=rgroups,
            ins=[v_cc_in.opt()], outs=[vg.opt()])

        # ---------- pass-1 KV prefetch (no deps — overlaps Q-proj) ------
        p1tiles = {}

        def load_p1(h):
            pk1 = kvs.tile([128, P1C], bf16, tag="pk1", name=f"pk1_{h}")
            nc.sync.dma_start(pk1, priorKT[h, :, 0:P1C])
            pv1 = kvs.tile([128, L1T, 130], bf16, tag="pv1", name=f"pv1_{h}")
            nc.sync.dma_start(pv1, priorVP[h, :, 0:L1T, :])
            p1tiles[h] = (pk1, pv1)

        load_p1(0)
        load_p1(1)

        # ---------- Q ----------
        raw_q, ccr_q, ssr_q = qk_projection(wq, bq_sb, gq_sb, bqg, "q")
        qn = acts.tile([128, NH, SC], bf16, tag="qn")

        # ---------- attention ----------
        part1 = outp.tile([128, NH, 3, 130], f32)
        oT = outp.tile([128, NH, SC], bf16)

        def attn_pass(h, k_tile, v_tile, n_tiles, phase):
            # Alternate 3-bank / 2-bank score-group tiles (A/B).  Software
            # pipeline: group g+1's scores are ISSUED BEFORE group g's AV
            # matmuls, so the in-order PE queue never stalls behind an
            # AV that waits on group g's exp.
            pa = pav.tile([128, 512], f32, tag="pav", name=f"po_{phase}_{h}")

            def emit_scores(g0, use_a):
                cap = 3 if use_a else 2
                gsz = min(cap, n_tiles - g0)
                sc_t = psc.tile([128, cap, 512], f32,
                                tag="scA" if use_a else "scB",
                                name=f"sc_{phase}_{h}_{g0}")
                for j in range(gsz):
                    lt = g0 + j
                    nc.tensor.matmul(sc_t[:, j, :SC],
                                     k_tile[:, lt * 128:(lt + 1) * 128],
                                     qn[:, h, :], start=True, stop=True)
                esc = escp.tile([128, 3, SC], bf16, tag="esc")
                if USE_SCH and not use_a:
                    # Schraudolph fast-exp on the Vector engine: bf16 bit
                    # pattern of ~exp(x*sm) is round(x*sm*128/ln2 + b) as
                    # int16.  C=8 centers the relative error so mixing
                    # with exact-exp l-tiles does not bias the softmax.
                    nc.vector.tensor_scalar(
                        esc[:, :gsz, :].bitcast(i16), sc_t[:, :gsz, :SC],
                        float(sm_scale * 128.0 / math.log(2.0)),
                        float(127.0 * 128.0 - 8.0),
                        Alu.mult, Alu.add)
                else:
                    nc.scalar.activation(esc[:, :gsz, :], sc_t[:, :gsz, :SC],
                                         Act.Exp, scale=constv(sm_scale))
                return esc, g0, gsz

            def emit_av(esc, g0, gsz):
                for j in range(gsz):
                    lt = g0 + j
                    for si in range(3):
                        # one bank holds all 3 accumulators; start=True
                        # clears has_written for the WHOLE bank, so only
                        # the first region's first matmul may issue it —
                        # the others overwrite-on-cleared-bit instead.
                        nc.tensor.matmul(
                            pa[:ST, si * 129:si * 129 + 129],
                            esc[:, j, si * ST:(si + 1) * ST],
                            v_tile[:, lt, 0:129],
                            start=(lt == 0 and si == 0),
                            stop=(lt == n_tiles - 1))

            pending = []
            g0 = 0
            use_a = True
            while g0 < n_tiles:
                cur = emit_scores(g0, use_a)
                pending.append(cur)
                if len(pending) > 2:
                    emit_av(*pending.pop(0))
                g0 += cur[2]
                use_a = not use_a
            for p in pending:
                emit_av(*p)
            # drain psum -> part1 (copy on pass 1, add on pass 2)
            src = pa[0:ST, 0:387].rearrange("p (a b) -> p a b", a=3)
            if phase == "p":
                nc.vector.tensor_copy(part1[:ST, h, 0:3, 0:129], src)
            else:
                nc.vector.tensor_add(part1[:ST, h, 0:3, 0:129], src,
                                     part1[:ST, h, 0:3, 0:129])

        # pass 1: pure-prior l-tiles (overlaps the AllGathers); q-rope for
        # head h+1 runs on DVE under head h's PE work
        for h in range(NH):
            rope_chunk(raw_q, ccr_q, ssr_q, h, qn[:, h, :], "q")
            if h + 2 < NH:
                load_p1(h + 2)
            pk1, pv1 = p1tiles.pop(h)
            attn_pass(h, pk1, pv1, L1T, "p")

        # divide by corrected denominator; transpose to [d, t]
        def finalize(h):
            for si in range(3):
                den = smal.tile([128, 1], f32, tag="den")
                nc.vector.tensor_scalar_add(den[:ST, :],
                                            part1[:ST, h, si, 128:129],
                                            -float(NPAD))
                rcp = smal.tile([128, 1], f32, tag="rcp")
                nc.vector.reciprocal(rcp[:ST, :], den[:ST, :])
                odiv = sqp.tile([128, 128], f32, tag="odiv")
                nc.vector.tensor_scalar_mul(odiv[:ST, :],
                                            part1[:ST, h, si, 0:128],
                                            rcp[:ST, 0:1])
                ptr = pb.tile([128, 512], f32, tag="pb", name=f"ptr_{h}_{si}")
                nc.tensor.transpose(ptr[:, :ST], odiv[:ST, :],
                                    ident[:ST, :ST])
                nc.vector.tensor_copy(oT[:, h, si * ST:(si + 1) * ST],
                                      ptr[:, :ST])

        # pass 2: prior tail + gathered current + pad
        for h in range(NH):
            pk2 = kvs.tile([128, P2C], bf16, tag="pk2")
            pv2 = kvs.tile([128, L2T, 130], bf16, tag="pv2")
            if NPAD:
                # zero the last l-tile's V rows up front; the gathered-V
                # DMAs below overwrite the valid rows (pad rows stay 0)
                nc.vector.memset(pv2[:, L2T - 1, 0:HD], 0.0)
            if TAIL:
                nc.sync.dma_start(pk2[:, 0:TAIL], priorKT[h, :, P1C:n_prior])
                nc.sync.dma_start(pv2[0:TAIL, 0, 0:HD],
                                  priorVP[h, 0:TAIL, L1T, 0:HD])
            for c in range(NC):
                nc.sync.dma_start(
                    pk2[:, TAIL + c * SC:TAIL + (c + 1) * SC],
                    kg[c, h].rearrange("(d t) -> d t", d=128))
                vsrc = vg[c, h].rearrange("(t d) -> t d", t=SC)
                # scatter 330 rows into the dense (p, lt) layout
                r0 = TAIL + c * SC
                t0 = 0
                while t0 < SC:
                    r = r0 + t0
                    p0 = r % 128
                    seg = min(128 - p0, SC - t0)
                    nc.sync.dma_start(
                        pv2[p0:p0 + seg, r // 128, 0:HD],
                        vsrc[t0:t0 + seg, :])
                    t0 += seg
            if NPAD:
                pr0 = TAIL + NC * SC
                assert pr0 // 128 == L2T - 1 and pr0 % 128 + NPAD == 128
                nc.vector.memset(pk2[:, pr0:P2C], 0.0)
            nc.vector.memset(pv2[:, :, 128:129], 1.0)
            attn_pass(h, pk2, pv2, L2T, "c")
            # finalize head h-1 now: its divide/transpose chain queues
            # BEHIND head h's attention on the in-order PE queue, so the
            # DVE dependency latency never stalls the next head's scores
            if h > 0:
                finalize(h - 1)
        finalize(NH - 1)

        # ---------- output projection ----------
        for oc in range(3):
            pot = psc.tile([128, 3, 512], f32, tag="scA", name=f"pout_{oc}")
            for h in range(NH):
                wc = wstr.tile([128, 512], bf16, tag="wc", name=f"wo_{oc}_{h}")
                nc.sync.dma_start(wc, wo[:, h, oc])
                for tci in range(3):
                    nc.tensor.matmul(
                        pot[:ST, tci, :],
                        oT[:, h, tci * ST:(tci + 1) * ST], wc,
                        start=(h == 0), stop=False)
            for tci in range(3):
                nc.tensor.matmul(
                    pot[:ST, tci, :], ones_row[:, :ST],
                    bo_sb[:, oc * 512:(oc + 1) * 512],
                    start=False, stop=True)
                ob = sqp.tile([128, 512], f32, tag="ob")
                nc.vector.tensor_copy(ob[:ST, :], pot[:ST, tci, :])
                nc.sync.dma_start(
                    out[tci * ST:(tci + 1) * ST, oc * 512:(oc + 1) * 512],
                    ob[:ST, :])

    nc.compile()
    return nc


def _prep(inputs):
    x = np.asarray(inputs["x"], np.float32)
    freqs_angle = np.asarray(inputs["freqs_angle"], np.float32)
    prior_k = np.asarray(inputs["prior_k"], np.float32)
    prior_v = np.asarray(inputs["prior_v"], np.float32)
    cs = int(np.asarray(inputs["current_start"]))

    block = 3 * FRAME
    block_end = (cs // block + 1) * block
    keep_from = max(0, block_end - 6 * FRAME)
    keep_size = min(cs + S_TOTAL - keep_from, prior_k.shape[0] + S_TOTAL)
    n_prior = keep_size - S_TOTAL
    p0 = prior_k.shape[0] - n_prior
    l_pad = -(-(n_prior + S_TOTAL) // 128) * 128
    l1t = n_prior // 128

    perm = np.concatenate(
        [h * HD + np.concatenate([np.arange(0, HD, 2), np.arange(1, HD, 2)])
         for h in range(NH)])

    def qk_w(w):
        wt = np.ascontiguousarray(np.asarray(w, np.float32)[perm].T)
        return np.ascontiguousarray(
            wt.reshape(NK, 128, NH, 128).transpose(1, 2, 0, 3)).astype(_BF16)

    def vo_w(w):
        wt = np.ascontiguousarray(np.asarray(w, np.float32).T)
        return np.ascontiguousarray(
            wt.reshape(NK, 128, 3, 512).transpose(1, 0, 2, 3)).astype(_BF16)

    WqP = qk_w(inputs["Wq"])
    WkP = qk_w(inputs["Wk"])
    WvP = vo_w(inputs["Wv"])
    WoP = vo_w(inputs["Wo"])

    def two(vec, p=None):
        v = np.asarray(vec, np.float32)
        if p is not None:
            v = v[p]
        return np.ascontiguousarray(v.reshape(NH, HD).T)

    bq2 = two(inputs["bq"], perm)
    bk2 = two(inputs["bk"], perm)
    gq2 = two(inputs["gq"], perm)
    gk2 = two(inputs["gk"], perm)
    bv1 = np.asarray(inputs["bv"], np.float32).reshape(1, DIM).astype(_BF16)
    bo1 = np.asarray(inputs["bo"], np.float32).reshape(1, DIM).astype(_BF16)

    pswT = np.zeros((HD, HD), _BF16)
    for r in range(HD):
        pswT[(r + HALF) % HD, r] = 1.0   # lhsT of the half-swap permutation

    theta = _build_theta(freqs_angle, cs)          # [S, 64]
    thetaT = np.ascontiguousarray(theta.T)

    pk = prior_k[p0:].reshape(n_prior, DIM)[:, perm]
    priorKT = np.ascontiguousarray(
        pk.T.reshape(NH, HD, n_prior)).astype(_BF16)

    # prior V laid as [h, p, lt, 130] with row index lt*128 + p; cols
    # 128:130 are the softmax-denominator ones column (host-prefilled)
    pv_kept = prior_v[p0:]                          # [n_prior, NH, HD]
    priorVP = np.zeros((NH, 128, l1t + 1, 130), np.float32)
    priorVP[:, :, :, 128:130] = 1.0
    full = pv_kept[:l1t * 128].reshape(l1t, 128, NH, HD)
    priorVP[:, :, :l1t, 0:HD] = full.transpose(2, 1, 0, 3)
    tail = n_prior - l1t * 128
    if tail:
        priorVP[:, 0:tail, l1t, 0:HD] = pv_kept[l1t * 128:].transpose(1, 0, 2)
    priorVP = priorVP.astype(_BF16)

    xT = np.ascontiguousarray(x[0].T).astype(_BF16)              # [DIM, S]

    shared = dict(wq=WqP, wk=WkP, wv=WvP, wo=WoP, bq2=bq2, bk2=bk2,
                  gq2=gq2, gk2=gk2, bv1=bv1, bo1=bo1, pswT=pswT,
                  priorKT=priorKT, priorVP=priorVP)
    in_maps = []
    for c in range(NC):
        m = dict(shared)
        xc = np.ascontiguousarray(xT[:, c * SC:(c + 1) * SC])
        m["xs_d"] = np.ascontiguousarray(
            xc.reshape(NK, 128, SC).transpose(1, 0, 2))
        m["thetaT"] = np.ascontiguousarray(thetaT[:, c * SC:(c + 1) * SC])
        in_maps.append(m)
    return in_maps, (n_prior, l_pad)


def kernel(**inputs) -> np.ndarray:
    import os
    from concourse.bass_utils import run_bass_kernel_spmd

    in_maps, key = _prep(inputs)
    if key not in _cache:
        _cache[key] = _build_program(*key)
    nc = _cache[key]

    trace = bool(int(os.environ.get("KERNEL_TRACE", "0")))
    try:
        res = run_bass_kernel_spmd(
            nc, in_maps, core_ids=list(range(NC)), trace=trace,
            trace_cores=list(range(NC)) if trace else None)
    except ModuleNotFoundError:
        res = run_bass_kernel_spmd(nc, in_maps, core_ids=list(range(NC)),
                                   trace=False)
    kernel.last_results = res
    outp = np.concatenate([res.results[c]["out"] for c in range(NC)], axis=0)
    return outp[None].astype(np.float32)


# revision 42
# speedup vs baseline: 1.1188x; 1.1188x over previous
"""Trainium2 Bass kernel for nn_CausalSelfAttention_5411658793445.

Sharding: queries (token dim) split 8 ways; K/V projection also token-split,
with the current block's roped K / V exchanged via one fused AllGather so
every core attends over the full kept KV window.

v2 changes vs the 607us baseline (trace-driven):
  - exp batching: score l-tiles are matmul'd into a 5-bank PSUM tile and
    exponentiated by ONE ACT instruction over a strided [128, 5, 330] AP.
    The ACT engine has a ~352-cycle fixed cost per instruction; the baseline
    paid it per 330-element tile (579 ns avg, 313 us total).  Grouped, the
    amortized cost is ~400 ns/tile.
  - dense KV packing: kept KV = prior tail (2640) + current (2640) = 5280,
    padded to 5376 (42 l-tiles) instead of the baseline's 5760 (-6.7% PE/ACT
    work in attention, -14% collective bytes).
  - one fused AllGather for K and V instead of two.
  - host-side weight / prior-KV re-layout so every big DMA is one
    contiguous descriptor per partition.
  - divides moved from ACT (Act.Copy w/ scale) to the Vector engine.
"""

import math
from contextlib import ExitStack

import numpy as np
import ml_dtypes

NC = 8
DIM, NH, HD = 1536, 12, 128
HALF = 64
H, W = 22, 40
FRAME = H * W            # 880
S_TOTAL = 2640
SC = S_TOTAL // NC       # 330
ST = 110                 # s-subtile (330 = 3*110)
EPS = 1e-6
CT = 22
CH = 21
CW = 21
NK = DIM // 128          # 12 contraction chunks
USE_SCH = True           # Vector-engine Schraudolph exp for B-groups

_BF16 = ml_dtypes.bfloat16
_cache: dict = {}


def _build_theta(freqs_angle, cs):
    start_frame = cs // FRAME
    nf = S_TOTAL // FRAME
    t = freqs_angle[start_frame:start_frame + nf, :CT]
    h = freqs_angle[:H, CT:CT + CH]
    w = freqs_angle[:W, CT + CH:CT + CH + CW]
    tf = np.broadcast_to(t[:, None, None, :], (nf, H, W, CT))
    hf = np.broadcast_to(h[None, :, None, :], (nf, H, W, CH))
    wf = np.broadcast_to(w[None, None, :, :], (nf, H, W, CW))
    return np.concatenate([tf, hf, wf], axis=-1).reshape(nf * H * W, HALF)


def _build_program(n_prior, l_pad):
    import concourse.bass as bass  # noqa: F401
    import concourse.tile as tile
    from concourse import bacc, mybir
    from concourse.masks import make_identity

    f32 = mybir.dt.float32
    bf16 = mybir.dt.bfloat16
    i16 = mybir.dt.int16
    Act = mybir.ActivationFunctionType
    Alu = mybir.AluOpType

    NLT = l_pad // 128                    # total l-tiles (42)
    L1T = n_prior // 128                  # pure-prior l-tiles (20)
    P1C = L1T * 128                       # pass-1 cols (2560)
    TAIL = n_prior - P1C                  # prior tail cols in pass 2 (80)
    P2C = l_pad - P1C                     # pass-2 cols (2816)
    L2T = P2C // 128                      # pass-2 l-tiles (22)
    NPAD = l_pad - (n_prior + S_TOTAL)    # zero-pad kv rows (96)
    sm_scale = 1.0 / math.sqrt(HD)
    CCN = HD * SC                         # flat K or V block per head (42240)

    nc = bacc.Bacc("TRN2", target_bir_lowering=False, debug=False,
                   num_devices=NC)

    # host-prepped layouts: every big DMA is contiguous per partition
    xs_d = nc.dram_tensor("xs_d", [128, NK, SC], bf16, kind="ExternalInput").ap()
    thetaT = nc.dram_tensor("thetaT", [HALF, SC], f32, kind="ExternalInput").ap()
    wq = nc.dram_tensor("wq", [128, NH, NK, 128], bf16, kind="ExternalInput").ap()
    wk = nc.dram_tensor("wk", [128, NH, NK, 128], bf16, kind="ExternalInput").ap()
    wv = nc.dram_tensor("wv", [128, NK, 3, 512], bf16, kind="ExternalInput").ap()
    wo = nc.dram_tensor("wo", [128, NK, 3, 512], bf16, kind="ExternalInput").ap()
    bq2 = nc.dram_tensor("bq2", [HD, NH], f32, kind="ExternalInput").ap()
    bk2 = nc.dram_tensor("bk2", [HD, NH], f32, kind="ExternalInput").ap()
    gq2 = nc.dram_tensor("gq2", [HD, NH], f32, kind="ExternalInput").ap()
    gk2 = nc.dram_tensor("gk2", [HD, NH], f32, kind="ExternalInput").ap()
    bv1 = nc.dram_tensor("bv1", [1, DIM], bf16, kind="ExternalInput").ap()
    bo1 = nc.dram_tensor("bo1", [1, DIM], bf16, kind="ExternalInput").ap()
    pswT = nc.dram_tensor("pswT", [HD, HD], bf16, kind="ExternalInput").ap()
    priorKT = nc.dram_tensor("priorKT", [NH, HD, n_prior], bf16,
                             kind="ExternalInput").ap()
    # prior V pre-laid as [h, p, lt, 130]  (lt*128+p = kept row index;
    # cols 128:130 pre-filled 1.0 so the load is one run per partition)
    priorVP = nc.dram_tensor("priorVP", [NH, 128, L1T + 1, 130], bf16,
                             kind="ExternalInput").ap()
    out = nc.dram_tensor("out", [SC, DIM], f32, kind="ExternalOutput").ap()

    with tile.TileContext(nc, trace_sim=False) as tc, ExitStack() as ctx:
        consts = ctx.enter_context(tc.tile_pool(name="consts", bufs=1))
        wstr = ctx.enter_context(tc.tile_pool(name="wstr", bufs=3))
        xpool = ctx.enter_context(tc.tile_pool(name="xpool", bufs=1))
        acts = ctx.enter_context(tc.tile_pool(name="acts", bufs=1))
        sqp = ctx.enter_context(tc.tile_pool(name="sqp", bufs=2))
        csrp = ctx.enter_context(tc.tile_pool(name="csrp", bufs=2))
        kvs = ctx.enter_context(tc.tile_pool(name="kvs", bufs=2))
        escp = ctx.enter_context(tc.tile_pool(name="escp", bufs=4))
        smal = ctx.enter_context(tc.tile_pool(name="smal", bufs=4))
        outp = ctx.enter_context(tc.tile_pool(name="outp", bufs=1))
        dram = ctx.enter_context(tc.tile_pool(name="dram", bufs=1, space="DRAM"))
        # PSUM: psc (scA 3 banks + scB 2 banks) + pav (2x1) + pb (1) = 8
        psc = ctx.enter_context(tc.tile_pool(name="psc", bufs=1, space="PSUM"))
        pav = ctx.enter_context(tc.tile_pool(name="pav", bufs=2, space="PSUM"))
        pb = ctx.enter_context(tc.tile_pool(name="pb", bufs=1, space="PSUM"))

        # ---------- constants ----------
        _constv_cache = {}

        def constv(val):
            if val not in _constv_cache:
                t = consts.tile([128, 1], f32, name=f"cv_{len(_constv_cache)}")
                nc.vector.memset(t, val)
                _constv_cache[val] = t
            return _constv_cache[val]

        # x first: the first projection matmul depends only on this + wm0
        xs = xpool.tile([128, NK, SC], bf16)
        nc.sync.dma_start(xs, xs_d)

        ident = consts.tile([128, 128], f32)
        make_identity(nc, ident)
        ones_col = consts.tile([128, 1], f32)
        nc.vector.memset(ones_col, 1.0)
        ones_row = consts.tile([1, 128], bf16)
        nc.vector.memset(ones_row, 1.0)
        ones_row_f = consts.tile([1, 128], f32)
        nc.vector.memset(ones_row_f, 1.0)
        psw_sb = consts.tile([HD, HD], bf16)
        nc.sync.dma_start(psw_sb, pswT)
        th2 = consts.tile([128, SC], f32)
        nc.sync.dma_start(th2[0:HALF, :], thetaT)
        nc.sync.dma_start(th2[HALF:128, :], thetaT)
        # CC = [cos; cos], SS = [-sin; sin]
        cc = consts.tile([128, SC], f32)
        ss = consts.tile([128, SC], f32)
        nc.scalar.activation(cc, th2, Act.Sin, bias=constv(math.pi / 2.0))
        nc.scalar.activation(ss[0:HALF, :], th2[0:HALF, :], Act.Sin,
                             scale=constv(-1.0)[0:HALF])
        nc.scalar.activation(ss[HALF:128, :], th2[HALF:128, :], Act.Sin)
        bq_sb = consts.tile([HD, NH], f32)
        bk_sb = consts.tile([HD, NH], f32)
        gq_sb = consts.tile([HD, NH], f32)
        gk_sb = consts.tile([HD, NH], f32)
        nc.sync.dma_start(bq_sb, bq2)
        nc.sync.dma_start(bk_sb, bk2)
        nc.sync.dma_start(gq_sb, gq2)
        nc.sync.dma_start(gk_sb, gk2)
        bqg = consts.tile([HD, NH], f32)
        bkg = consts.tile([HD, NH], f32)
        nc.vector.tensor_mul(bqg, bq_sb, gq_sb)
        nc.vector.tensor_mul(bkg, bk_sb, gk_sb)
        bv_sb = consts.tile([1, DIM], bf16)
        bo_sb = consts.tile([1, DIM], bf16)
        nc.sync.dma_start(bv_sb, bv1)
        nc.sync.dma_start(bo_sb, bo1)

        # ---------- internal DRAM for the K / V collectives ----------
        k_cc_in = dram.tile([NH, CCN], bf16)
        v_cc_in = dram.tile([NH, CCN], bf16)
        kg = dram.tile([NC, NH, CCN], bf16, addr_space="Shared")
        vg = dram.tile([NC, NH, CCN], bf16, addr_space="Shared")
        rgroups = [list(range(NC))]

        # ---------- projection helper (q / k): [d, t] + norm factors ------
        def qk_projection(w_dram, b_sb, g_sb, bg_sb, name):
            raw = acts.tile([128, NH, SC], bf16, tag=f"raw_{name}")
            pss = pb.tile([128, SC], f32, tag="pb", name=f"pss_{name}")
            for m in range(NH):
                wm = wstr.tile([128, NK, 128], bf16, tag="wm",
                               name=f"wm_{name}_{m}")
                nc.sync.dma_start(wm, w_dram[:, m])
                ps = pav.tile([128, 512], f32, tag="pav", name=f"pj_{name}_{m}")
                for kk in range(NK):
                    nc.tensor.matmul(
                        ps[:, :SC], wm[:, kk, :], xs[:, kk, :],
                        start=(kk == 0), stop=(kk == NK - 1))
                nc.scalar.activation(raw[:, m, :], ps[:, :SC], Act.Identity,
                                     bias=bg_sb[:, m:m + 1],
                                     scale=g_sb[:, m:m + 1])
                sq = sqp.tile([128, SC], f32, tag="sq")
                nc.scalar.activation(sq, ps[:, :SC], Act.Square,
                                     bias=b_sb[:, m:m + 1])
                nc.tensor.matmul(pss[0:1, :], ones_col, sq,
                                 start=(m == 0), stop=(m == NH - 1))
            r1 = smal.tile([1, SC], f32, tag="r1")
            nc.scalar.activation(r1, pss[0:1, :], Act.Sqrt,
                                 scale=constv(1.0 / DIM)[0:1],
                                 bias=constv(EPS)[0:1])
            rr = smal.tile([1, SC], f32, tag="rr")
            nc.vector.reciprocal(rr, r1)
            rrb = psc.tile([128, 512], f32, tag="scB", name=f"rrb_{name}")
            nc.tensor.matmul(rrb[:, :SC], ones_row_f, rr,
                             start=True, stop=True)
            ccr = csrp.tile([128, SC], f32, tag="ccr")
            ssr = csrp.tile([128, SC], f32, tag="ssr")
            nc.vector.tensor_mul(ccr, cc, rrb[:, :SC])
            nc.vector.tensor_mul(ssr, ss, rrb[:, :SC])
            return raw, ccr, ssr

        def rope_chunk(raw, ccr, ssr, m, dst_ap, name):
            # dst = raw*ccr + swap_halves(raw)*ssr   (swap via PE matmul)
            pw = pb.tile([128, 512], f32, tag="pb", name=f"sw_{name}_{m}")
            nc.tensor.matmul(pw[:, :SC], psw_sb, raw[:, m, :],
                             start=True, stop=True)
            m1 = sqp.tile([128, SC], f32, tag="m1")
            nc.vector.tensor_mul(m1, raw[:, m, :], ccr)
            m2 = sqp.tile([128, SC], f32, tag="m2")
            nc.vector.tensor_mul(m2, pw[:, :SC], ssr)
            nc.vector.tensor_add(dst_ap, m1, m2)

        # ---------- K ----------
        raw_k, ccr_k, ssr_k = qk_projection(wk, bk_sb, gk_sb, bkg, "k")
        kn = acts.tile([128, NH, SC], bf16, tag="kn")
        for m in range(NH):
            rope_chunk(raw_k, ccr_k, ssr_k, m, kn[:, m, :], "k")
        for m in range(NH):
            nc.sync.dma_start(
                k_cc_in[m].rearrange("(d t) -> d t", d=128), kn[:, m, :])
        nc.gpsimd.collective_compute(
            "AllGather", Alu.bypass, replica_groups=rgroups,
            ins=[k_cc_in.opt()], outs=[kg.opt()])

        # ---------- V (direct [t, d] production) ----------
        vt = acts.tile([128, 3, DIM], bf16, tag="vt")
        for oc in range(3):
            pvt = psc.tile([128, 3, 512], f32, tag="scA", name=f"pv_{oc}")
            for kk in range(NK):
                wc = wstr.tile([128, 512], bf16, tag="wc", name=f"wv_{oc}_{kk}")
                nc.sync.dma_start(wc, wv[:, kk, oc])
                for tci in range(3):
                    nc.tensor.matmul(
                        pvt[:ST, tci, :],
                        xs[:, kk, tci * ST:(tci + 1) * ST], wc,
                        start=(kk == 0), stop=False)
            for tci in range(3):
                nc.tensor.matmul(
                    pvt[:ST, tci, :], ones_row[:, :ST],
                    bv_sb[:, oc * 512:(oc + 1) * 512],
                    start=False, stop=True)
                nc.vector.tensor_copy(
                    vt[:ST, tci, oc * 512:(oc + 1) * 512], pvt[:ST, tci, :])
        for h in range(NH):
            nc.sync.dma_start(
                v_cc_in[h].rearrange("(tc p d) -> p tc d", p=ST, d=HD),
                vt[:ST, :, h * 128:(h + 1) * 128])
        nc.gpsimd.collective_compute(
            "AllGather", Alu.bypass, replica_groups=rgroups,
            ins=[v_cc_in.opt()], outs=[vg.opt()])

        # ---------- pass-1 KV prefetch (no deps — overlaps Q-proj) ------
        p1tiles = {}

        def load_p1(h):
            pk1 = kvs.tile([128, P1C], bf16, tag="pk1", name=f"pk1_{h}")
            nc.sync.dma_start(pk1, priorKT[h, :, 0:P1C])
            pv1 = kvs.tile([128, L1T, 130], bf16, tag="pv1", name=f"pv1_{h}")
            nc.sync.dma_start(pv1, priorVP[h, :, 0:L1T, :])
            p1tiles[h] = (pk1, pv1)

        load_p1(0)
        load_p1(1)

        # ---------- Q ----------
        raw_q, ccr_q, ssr_q = qk_projection(wq, bq_sb, gq_sb, bqg, "q")
        qn = acts.tile([128, NH, SC], bf16, tag="qn")

        # ---------- attention ----------
        part1 = outp.tile([128, NH, 3, 130], f32)
        oT = outp.tile([128, NH, SC], bf16)

        def attn_pass(h, k_tile, v_tile, n_tiles, phase):
            # Alternate 3-bank / 2-bank score-group tiles (A/B).  Software
            # pipeline: group g+1's scores are ISSUED BEFORE group g's AV
            # matmuls, so the in-order PE queue never stalls behind an
            # AV that waits on group g's exp.
            pa = pav.tile([128, 512], f32, tag="pav", name=f"po_{phase}_{h}")

            def emit_scores(g0, use_a):
                cap = 3 if use_a else 2
                gsz = min(cap, n_tiles - g0)
                sc_t = psc.tile([128, cap, 512], f32,
                                tag="scA" if use_a else "scB",
                                name=f"sc_{phase}_{h}_{g0}")
                for j in range(gsz):
                    lt = g0 + j
                    nc.tensor.matmul(sc_t[:, j, :SC],
                                     k_tile[:, lt * 128:(lt + 1) * 128],
                                     qn[:, h, :], start=True, stop=True)
                esc = escp.tile([128, 3, SC], bf16, tag="esc")
                if USE_SCH and not use_a:
                    # Schraudolph fast-exp on the Vector engine: bf16 bit
                    # pattern of ~exp(x*sm) is round(x*sm*128/ln2 + b) as
                    # int16.  C=8 centers the relative error so mixing
                    # with exact-exp l-tiles does not bias the softmax.
                    nc.vector.tensor_scalar(
                        esc[:, :gsz, :].bitcast(i16), sc_t[:, :gsz, :SC],
                        float(sm_scale * 128.0 / math.log(2.0)),
                        float(127.0 * 128.0 - 8.0),
                        Alu.mult, Alu.add)
                else:
                    nc.scalar.activation(esc[:, :gsz, :], sc_t[:, :gsz, :SC],
                                         Act.Exp, scale=constv(sm_scale))
                return esc, g0, gsz

            def emit_av(esc, g0, gsz):
                for j in range(gsz):
                    lt = g0 + j
                    for si in range(3):
                        # one bank holds all 3 accumulators; start=True
                        # clears has_written for the WHOLE bank, so only
                        # the first region's first matmul may issue it —
                        # the others overwrite-on-cleared-bit instead.
                        nc.tensor.matmul(
                            pa[:ST, si * 129:si * 129 + 129],
                            esc[:, j, si * ST:(si + 1) * ST],
                            v_tile[:, lt, 0:129],
                            start=(lt == 0 and si == 0),
                            stop=(lt == n_tiles - 1))

            pending = []
            g0 = 0
            use_a = True
            while g0 < n_tiles:
                cur = emit_scores(g0, use_a)
                pending.append(cur)
                if len(pending) > 2:
                    emit_av(*pending.pop(0))
                g0 += cur[2]
                use_a = not use_a
            for p in pending:
                emit_av(*p)
            # drain psum -> part1 (copy on pass 1, add on pass 2)
            src = pa[0:ST, 0:387].rearrange("p (a b) -> p a b", a=3)
            if phase == "p":
                nc.vector.tensor_copy(part1[:ST, h, 0:3, 0:129], src)
            else:
                nc.vector.tensor_add(part1[:ST, h, 0:3, 0:129], src,
                                     part1[:ST, h, 0:3, 0:129])

        # pass 1: pure-prior l-tiles (overlaps the AllGathers).  q-rope
        # runs TWO heads ahead so its DVE chain clears the in-order DVE
        # queue well before head h's scores need qn[:, h]
        rope_chunk(raw_q, ccr_q, ssr_q, 0, qn[:, 0, :], "q")
        rope_chunk(raw_q, ccr_q, ssr_q, 1, qn[:, 1, :], "q")
        for h in range(NH):
            if h + 2 < NH:
                rope_chunk(raw_q, ccr_q, ssr_q, h + 2, qn[:, h + 2, :], "q")
                load_p1(h + 2)
            pk1, pv1 = p1tiles.pop(h)
            attn_pass(h, pk1, pv1, L1T, "p")

        # divide by corrected denominator; transpose to [d, t]
        def finalize(h):
            for si in range(3):
                den = smal.tile([128, 1], f32, tag="den")
                nc.vector.tensor_scalar_add(den[:ST, :],
                                            part1[:ST, h, si, 128:129],
                                            -float(NPAD))
                rcp = smal.tile([128, 1], f32, tag="rcp")
                nc.vector.reciprocal(rcp[:ST, :], den[:ST, :])
                odiv = sqp.tile([128, 128], f32, tag="odiv")
                nc.vector.tensor_scalar_mul(odiv[:ST, :],
                                            part1[:ST, h, si, 0:128],
                                            rcp[:ST, 0:1])
                ptr = pb.tile([128, 512], f32, tag="pb", name=f"ptr_{h}_{si}")
                nc.tensor.transpose(ptr[:, :ST], odiv[:ST, :],
                                    ident[:ST, :ST])
                nc.vector.tensor_copy(oT[:, h, si * ST:(si + 1) * ST],
                                      ptr[:, :ST])

        # pass 2: prior tail + gathered current + pad
        for h in range(NH):
            pk2 = kvs.tile([128, P2C], bf16, tag="pk2")
            pv2 = kvs.tile([128, L2T, 130], bf16, tag="pv2")
            if NPAD:
                # zero the last l-tile's V rows up front; the gathered-V
                # DMAs below overwrite the valid rows (pad rows stay 0)
                nc.vector.memset(pv2[:, L2T - 1, 0:HD], 0.0)
            if TAIL:
                nc.sync.dma_start(pk2[:, 0:TAIL], priorKT[h, :, P1C:n_prior])
                nc.sync.dma_start(pv2[0:TAIL, 0, 0:HD],
                                  priorVP[h, 0:TAIL, L1T, 0:HD])
            for c in range(NC):
                nc.sync.dma_start(
                    pk2[:, TAIL + c * SC:TAIL + (c + 1) * SC],
                    kg[c, h].rearrange("(d t) -> d t", d=128))
                vsrc = vg[c, h].rearrange("(t d) -> t d", t=SC)
                # scatter 330 rows into the dense (p, lt) layout
                r0 = TAIL + c * SC
                t0 = 0
                while t0 < SC:
                    r = r0 + t0
                    p0 = r % 128
                    seg = min(128 - p0, SC - t0)
                    nc.sync.dma_start(
                        pv2[p0:p0 + seg, r // 128, 0:HD],
                        vsrc[t0:t0 + seg, :])
                    t0 += seg
            if NPAD:
                pr0 = TAIL + NC * SC
                assert pr0 // 128 == L2T - 1 and pr0 % 128 + NPAD == 128
                nc.vector.memset(pk2[:, pr0:P2C], 0.0)
            nc.vector.memset(pv2[:, :, 128:129], 1.0)
            attn_pass(h, pk2, pv2, L2T, "c")
            # finalize head h-1 now: its divide/transpose chain queues
            # BEHIND head h's attention on the in-order PE queue, so the
            # DVE dependency latency never stalls the next head's scores
            if h > 0:
                finalize(h - 1)
        finalize(NH - 1)

        # ---------- output projection ----------
        for oc in range(3):
            pot = psc.tile([128, 3, 512], f32, tag="scA", name=f"pout_{oc}")
            for h in range(NH):
                wc = wstr.tile([128, 512], bf16, tag="wc", name=f"wo_{oc}_{h}")
                nc.sync.dma_start(wc, wo[:, h, oc])
                for tci in range(3):
                    nc.tensor.matmul(
                        pot[:ST, tci, :],
                        oT[:, h, tci * ST:(tci + 1) * ST], wc,
                        start=(h == 0), stop=False)
            for tci in range(3):
                nc.tensor.matmul(
                    pot[:ST, tci, :], ones_row[:, :ST],
                    bo_sb[:, oc * 512:(oc + 1) * 512],
                    start=False, stop=True)
                ob = sqp.tile([128, 512], f32, tag="ob")
                nc.vector.tensor_copy(ob[:ST, :], pot[:ST, tci, :])
                nc.sync.dma_start(
                    out[tci * ST:(tci + 1) * ST, oc * 512:(oc + 1) * 512],
                    ob[:ST, :])

    nc.compile()
    return nc


def _prep(inputs):
    x = np.asarray(inputs["x"], np.float32)
    freqs_angle = np.asarray(inputs["freqs_angle"], np.float32)
    prior_k = np.asarray(inputs["prior_k"], np.float32)
    prior_v = np.asarray(inputs["prior_v"], np.float32)
    cs = int(np.asarray(inputs["current_start"]))

    block = 3 * FRAME
    block_end = (cs // block + 1) * block
    keep_from = max(0, block_end - 6 * FRAME)
    keep_size = min(cs + S_TOTAL - keep_from, prior_k.shape[0] + S_TOTAL)
    n_prior = keep_size - S_TOTAL
    p0 = prior_k.shape[0] - n_prior
    l_pad = -(-(n_prior + S_TOTAL) // 128) * 128
    l1t = n_prior // 128

    perm = np.concatenate(
        [h * HD + np.concatenate([np.arange(0, HD, 2), np.arange(1, HD, 2)])
         for h in range(NH)])

    def qk_w(w):
        wt = np.ascontiguousarray(np.asarray(w, np.float32)[perm].T)
        return np.ascontiguousarray(
            wt.reshape(NK, 128, NH, 128).transpose(1, 2, 0, 3)).astype(_BF16)

    def vo_w(w):
        wt = np.ascontiguousarray(np.asarray(w, np.float32).T)
        return np.ascontiguousarray(
            wt.reshape(NK, 128, 3, 512).transpose(1, 0, 2, 3)).astype(_BF16)

    WqP = qk_w(inputs["Wq"])
    WkP = qk_w(inputs["Wk"])
    WvP = vo_w(inputs["Wv"])
    WoP = vo_w(inputs["Wo"])

    def two(vec, p=None):
        v = np.asarray(vec, np.float32)
        if p is not None:
            v = v[p]
        return np.ascontiguousarray(v.reshape(NH, HD).T)

    bq2 = two(inputs["bq"], perm)
    bk2 = two(inputs["bk"], perm)
    gq2 = two(inputs["gq"], perm)
    gk2 = two(inputs["gk"], perm)
    bv1 = np.asarray(inputs["bv"], np.float32).reshape(1, DIM).astype(_BF16)
    bo1 = np.asarray(inputs["bo"], np.float32).reshape(1, DIM).astype(_BF16)

    pswT = np.zeros((HD, HD), _BF16)
    for r in range(HD):
        pswT[(r + HALF) % HD, r] = 1.0   # lhsT of the half-swap permutation

    theta = _build_theta(freqs_angle, cs)          # [S, 64]
    thetaT = np.ascontiguousarray(theta.T)

    pk = prior_k[p0:].reshape(n_prior, DIM)[:, perm]
    priorKT = np.ascontiguousarray(
        pk.T.reshape(NH, HD, n_prior)).astype(_BF16)

    # prior V laid as [h, p, lt, 130] with row index lt*128 + p; cols
    # 128:130 are the softmax-denominator ones column (host-prefilled)
    pv_kept = prior_v[p0:]                          # [n_prior, NH, HD]
    priorVP = np.zeros((NH, 128, l1t + 1, 130), np.float32)
    priorVP[:, :, :, 128:130] = 1.0
    full = pv_kept[:l1t * 128].reshape(l1t, 128, NH, HD)
    priorVP[:, :, :l1t, 0:HD] = full.transpose(2, 1, 0, 3)
    tail = n_prior - l1t * 128
    if tail:
        priorVP[:, 0:tail, l1t, 0:HD] = pv_kept[l1t * 128:].transpose(1, 0, 2)
    priorVP = priorVP.astype(_BF16)

    xT = np.ascontiguousarray(x[0].T).astype(_BF16)              # [DIM, S]

    shared = dict(wq=WqP, wk=WkP, wv=WvP, wo=WoP, bq2=bq2, bk2=bk2,
                  gq2=gq2, gk2=gk2, bv1=bv1, bo1=bo1, pswT=pswT,
                  priorKT=priorKT, priorVP=priorVP)
    in_maps = []
    for c in range(NC):
        m = dict(shared)
        xc = np.ascontiguousarray(xT[:, c * SC:(c + 1) * SC])
        m["xs_d"] = np.ascontiguousarray(
            xc.reshape(NK, 128, SC).transpose(1, 0, 2))
        m["thetaT"] = np.ascontiguousarray(thetaT[:, c * SC:(c + 1) * SC])
        in_maps.append(m)
    return in_maps, (n_prior, l_pad)


def kernel(**inputs) -> np.ndarray:
    import os
    from concourse.bass_utils import run_bass_kernel_spmd

    in_maps, key = _prep(inputs)
    if key not in _cache:
        _cache[key] = _build_program(*key)
    nc = _cache[key]

    trace = bool(int(os.environ.get("KERNEL_TRACE", "0")))
    try:
        res = run_bass_kernel_spmd(
            nc, in_maps, core_ids=list(range(NC)), trace=trace,
            trace_cores=list(range(NC)) if trace else None)
    except ModuleNotFoundError:
        res = run_bass_kernel_spmd(nc, in_maps, core_ids=list(range(NC)),
                                   trace=False)
    kernel.last_results = res
    outp = np.concatenate([res.results[c]["out"] for c in range(NC)], axis=0)
    return outp[None].astype(np.float32)
